# revision 11
# baseline (speedup 1.0000x reference)
"""BitLinear (4-bit activation quant + ternary weight) Trainium2 kernel.

Full computation:
    xq  = round(clip(x / max_abs(x, row) * 7)) * max_abs / 7      (per-row 4-bit quant)
    wq  = sign_thresholded(w) * mean_abs(w, row)                   (ternary weight)
    out = xq @ wq.T + bias

Strategy (8 NeuronCores, data-parallel over rows of x):
  - Shard x rows 8 ways; replicate weight.
  - x and weight ship to the device as f16 (halves the HBM read traffic; the
    quant decisions from f16 inputs keep end-to-end rel err ~1e-2, inside the
    2e-2 gate). Output is exact f32 scaling of integer PE accumulations.
  - Matmul runs on exact small integers in fp8 (q in [-8,7], sign in
    {-1,0,1}) with DoubleRow perf mode. Rounding uses the +1.5*2^23
    magic-number trick == round-half-even.
  - Engine balance per 128-row s-tile (steady state): DVE absmax+scales and
    most column-scale multiplies; Pool the magic multiply-add and 1/3 of the
    column-scale multiplies; ACT both PSUM evictions (qt and out*rowscale)
    plus out-store DMA issues; PE transposes + matmuls at full clock.
"""

import os
import sys

os.environ.setdefault("MYCRO_LOCAL_CACHE", "1")

for _p in ("/opt/trn_rl_repo", "/root/.axon_site/_ro/trn_rl_repo"):
    if os.path.isdir(_p) and _p not in sys.path:
        sys.path.insert(0, _p)

import numpy as np

N_CORES = 8
S_SHARD = 4096
IN_F = 1024
OUT_F = 1024
P = 128
N_STILES = S_SHARD // P  # 32
N_KTILES = IN_F // P  # 8
N_OTILES = OUT_F // P  # 8
MM_N = 512
N_OHALF = OUT_F // MM_N  # 2
OUT_B = 4  # s-tiles per output store (2 MiB transfers)

MAGIC = 12582912.0
EPS = 1e-06

_prog_cache = {}


def _build_program(with_bias: bool):
    import concourse.bass as bass
    import concourse.mybir as mybir
    import concourse.tile as tile
    from concourse import bacc, bass_isa
    from concourse.masks import make_identity

    f32 = mybir.dt.float32
    f16 = mybir.dt.float16
    bf16 = mybir.dt.bfloat16
    f8 = mybir.dt.float8e4
    Alu = mybir.AluOpType
    Act = mybir.ActivationFunctionType

    nc = bacc.Bacc("TRN2", target_bir_lowering=False, debug=False)

    x_in = nc.dram_tensor("x_shard", [S_SHARD, IN_F], f16, kind="ExternalInput")
    w_in = nc.dram_tensor("weight", [OUT_F, IN_F], f16, kind="ExternalInput")
    if with_bias:
        b_in = nc.dram_tensor("bias", [OUT_F], f32, kind="ExternalInput")
    out_d = nc.dram_tensor("out", [S_SHARD, OUT_F], f32, kind="ExternalOutput")

    WPREP_S = int(os.environ.get("KWPREP", "8"))
    SIGN_PER_S = int(os.environ.get("KSIGNPS", "2"))
    LEAD = int(os.environ.get("KLEAD", "13"))
    XBUFS = int(os.environ.get("KXBUFS", "12"))
    OBUFS = int(os.environ.get("KOBUFS", "4"))

    with tile.TileContext(nc) as tc:
        from contextlib import ExitStack as _ES

        _wstack = _ES()
        with (
            tc.tile_pool(name="singles", bufs=1) as singles,
            tc.tile_pool(name="wtmp", bufs=2) as wtmp,
            tc.tile_pool(name="signp", bufs=2) as signp,
            tc.tile_pool(name="xp", bufs=XBUFS) as xp,
            tc.tile_pool(name="tp", bufs=3) as tp,
            tc.tile_pool(name="fevp", bufs=3) as fevp,
            tc.tile_pool(name="qtp", bufs=LEAD + 3) as qtp,
            tc.tile_pool(name="outp", bufs=OBUFS) as outp,
            tc.tile_pool(name="stats", bufs=8) as stats,
            tc.tile_pool(name="ma7p", bufs=N_STILES + 1) as ma7p,
            tc.tile_pool(name="tpsum", bufs=2, space="PSUM") as tpsum,
            tc.tile_pool(name="mpsum", bufs=2, space="PSUM") as mpsum,
            tc.tile_pool(name="dramp", bufs=1, space="DRAM") as dramp,
        ):
            identity = singles.tile([P, P], bf16)
            make_identity(nc, identity)
            identity_f = singles.tile([P, P], f32)
            make_identity(nc, identity_f)

            magneg = singles.tile([P, 1], f32)
            nc.vector.memset(magneg, -MAGIC)
            neg1 = singles.tile([P, 1], f32)
            nc.vector.memset(neg1, -1.0)

            signT8 = singles.tile([P, N_KTILES, OUT_F], f8)
            alpha_raw = singles.tile([P, N_OTILES], f32)

            wpool = _wstack.enter_context(tc.tile_pool(name="wpool", bufs=8))
            w_tiles = []
            for j in range(N_OTILES):
                w_t = wpool.tile([P, IN_F], f16, tag="w")
                w_tiles.append(w_t)

            def emit_wload(js):
                for j in js:
                    if j < N_OTILES:
                        nc.sync.dma_start(
                            out=w_tiles[j], in_=w_in[j * P : (j + 1) * P, :]
                        )

            def emit_wabs(j):
                nc.vector.tensor_reduce(
                    out=alpha_raw[:, j : j + 1],
                    in_=w_tiles[j],
                    axis=mybir.AxisListType.X,
                    op=Alu.add,
                    apply_absolute_value=True,
                )

            x_pairs = {}

            def emit_quant(s):
                if s % 2 == 0:
                    x2 = xp.tile([P, 2, IN_F], f16, tag="x")
                    if s == 0:
                        for g in range(2):
                            nc.sync.dma_start(
                                out=x2[:, g, :],
                                in_=x_in[(s + g) * P : (s + g + 1) * P, :],
                            )
                    else:
                        nc.sync.dma_start(
                            out=x2,
                            in_=x_in[s * P : (s + 2) * P, :].rearrange(
                                "(two p) f -> p two f", p=P
                            ),
                        )
                    x_pairs[s] = x2
                    x_t = x2[:, 0, :]
                else:
                    x_t = x_pairs.pop(s - 1)[:, 1, :]
                ma = stats.tile([P, 1], f32, tag="ma")
                nc.vector.tensor_reduce(
                    out=ma,
                    in_=x_t,
                    axis=mybir.AxisListType.X,
                    op=Alu.max,
                    apply_absolute_value=True,
                )
                ma7 = ma7p.tile([P, 1], f32, tag="ma7")
                nc.vector.tensor_scalar(
                    out=ma7,
                    in0=ma,
                    scalar1=float(1.0 / 7.0),
                    scalar2=float(EPS / 7.0),
                    op0=Alu.mult,
                    op1=Alu.max,
                )
                inv = stats.tile([P, 1], f32, tag="inv")
                nc.vector.reciprocal(out=inv, in_=ma7)
                # t = x*inv + MAGIC (f32; fraction now rounded half-to-even)
                # on DVE: all-SBUF tensor_scalar runs in 2x mode (~594 ns)
                t_t = tp.tile([P, IN_F], f32, tag="t")
                nc.vector.tensor_scalar(
                    out=t_t,
                    in0=x_t,
                    scalar1=inv,
                    scalar2=MAGIC,
                    op0=Alu.mult,
                    op1=Alu.add,
                )
                qt_ps = tpsum.tile([P, IN_F], f32, tag="tps")
                for k in range(N_KTILES):
                    nc.tensor.transpose(
                        qt_ps[:, k * P : (k + 1) * P],
                        t_t[:, k * P : (k + 1) * P],
                        identity_f,
                    )
                qt_sb = qtp.tile([P, N_KTILES, P], f8, tag="qt")
                nc.scalar.activation(
                    out=qt_sb.rearrange("p k c -> p (k c)"),
                    in_=qt_ps,
                    func=Act.Identity,
                    bias=magneg,
                    scale=1.0,
                )
                return ma7, qt_sb

            out_blocks = {}

            def emit_matmul(s, ma7, qt_sb):
                sb = (s // OUT_B) * OUT_B
                if s % OUT_B == 0:
                    ob = outp.tile([P, OUT_B, OUT_F], f32, tag="o")
                    out_blocks[sb] = ob
                else:
                    ob = out_blocks[sb]
                out_sb = ob[:, s % OUT_B, :]
                ps = mpsum.tile([P, OUT_F], f32, tag="mm")
                for h in range(N_OHALF):
                    for t in range(N_KTILES // 2):
                        nc.tensor.matmul(
                            ps[:, h * MM_N : (h + 1) * MM_N],
                            lhsT=qt_sb[:, 2 * t : 2 * t + 2, :],
                            rhs=signT8[
                                :, 2 * t : 2 * t + 2, h * MM_N : (h + 1) * MM_N
                            ],
                            start=(t == 0),
                            stop=(t == N_KTILES // 2 - 1),
                            perf_mode=mybir.MatmulPerfMode.DoubleRow,
                        )
                # out = (S * rowscale) * colscale: ACT evicts PSUM with the
                # per-row scale (GPSIMD can't read PSUM), then the per-column
                # multiply runs on DVE (2 of 3 tiles) or Pool (1 of 3).
                fev = fevp.tile([P, OUT_F], f32, tag="fev")
                nc.scalar.activation(
                    out=fev, in_=ps, func=Act.Identity, scale=ma7
                )
                nc.gpsimd.tensor_tensor(
                    out=out_sb, in0=fev, in1=colb, op=Alu.mult
                )
                if with_bias:
                    nc.gpsimd.tensor_tensor(
                        out=out_sb, in0=out_sb, in1=biasb, op=Alu.add
                    )
                if s % OUT_B == OUT_B - 1:
                    nc.scalar.dma_start(
                        out=out_d[sb * P : (s + 1) * P, :].rearrange(
                            "(b p) f -> p b f", p=P
                        ),
                        in_=out_blocks.pop(sb),
                    )

            def emit_wprep_head():
                # global threshold = 0.05 * mean(|w|)
                g0 = stats.tile([P, 1], f32, tag="g0")
                nc.vector.tensor_reduce(
                    out=g0, in_=alpha_raw, axis=mybir.AxisListType.X, op=Alu.add
                )
                g1 = stats.tile([P, 1], f32, tag="g1")
                nc.gpsimd.partition_all_reduce(
                    out_ap=g1, in_ap=g0, channels=P, reduce_op=bass_isa.ReduceOp.add
                )
                nc.vector.tensor_scalar(
                    out=thr,
                    in0=g1,
                    scalar1=float(0.05 / (OUT_F * IN_F)),
                    scalar2=None,
                    op0=Alu.mult,
                )
                nc.vector.tensor_scalar(
                    out=nthr, in0=thr, scalar1=-1.0, scalar2=None, op0=Alu.mult
                )
                nc.vector.tensor_scalar(
                    out=alpha_sb,
                    in0=alpha_raw,
                    scalar1=float(1.0 / IN_F),
                    scalar2=None,
                    op0=Alu.mult,
                )
                # column scale alpha broadcast to all partitions via DRAM bounce
                nc.sync.dma_start(
                    out=alpha_dram.rearrange("j p -> p j"), in_=alpha_sb
                )
                alpha_flat = alpha_dram.rearrange("j p -> (j p)")
                bcast_src = bass.AP(
                    tensor=alpha_flat.tensor,
                    offset=alpha_flat.offset,
                    ap=[[0, P]] + list(alpha_flat.ap),
                )
                nc.sync.dma_start(out=colb, in_=bcast_src)
                if with_bias:
                    bias_src = bass.AP(
                        tensor=b_in.tensor
                        if hasattr(b_in, "tensor")
                        else b_in[:].tensor,
                        offset=b_in[:].offset,
                        ap=[[0, P]] + list(b_in[:].ap),
                    )
                    nc.sync.dma_start(out=biasb, in_=bias_src)

            def emit_sign(j):
                # ternary sign: sign = (w >= thr) + (w > -thr) - 1.
                # The two comparisons are cheap 4x-mode DVE tensor_scalars;
                # the add happens for free in PSUM (accumulating transposes)
                # and the -1 rides the eviction bias.
                a_cmp = wtmp.tile([P, IN_F], bf16, tag="tmp")
                nc.vector.tensor_scalar(
                    out=a_cmp,
                    in0=w_tiles[j],
                    scalar1=nthr,
                    scalar2=None,
                    op0=Alu.is_gt,
                )
                b_cmp = signp.tile([P, IN_F], bf16, tag="sgn")
                nc.vector.tensor_scalar(
                    out=b_cmp,
                    in0=w_tiles[j],
                    scalar1=thr,
                    scalar2=None,
                    op0=Alu.is_ge,
                )
                ps = tpsum.tile([P, IN_F], f32, tag="tps")
                for k in range(N_KTILES):
                    nc.tensor.matmul(
                        ps[:, k * P : (k + 1) * P],
                        lhsT=a_cmp[:, k * P : (k + 1) * P],
                        rhs=identity,
                        start=True,
                        stop=False,
                    )
                    nc.tensor.matmul(
                        ps[:, k * P : (k + 1) * P],
                        lhsT=b_cmp[:, k * P : (k + 1) * P],
                        rhs=identity,
                        start=False,
                        stop=True,
                    )
                if j % 2 == 0:
                    nc.scalar.activation(
                        out=signT8[:, :, j * P : (j + 1) * P],
                        in_=ps.rearrange("p (k c) -> p k c", k=N_KTILES),
                        func=Act.Identity,
                        bias=neg1,
                    )
                else:
                    nc.vector.tensor_scalar(
                        out=signT8[:, :, j * P : (j + 1) * P],
                        in0=ps.rearrange("p (k c) -> p k c", k=N_KTILES),
                        scalar1=-1.0,
                        scalar2=None,
                        op0=Alu.add,
                    )

            thr = singles.tile([P, 1], f32)
            nthr = singles.tile([P, 1], f32)
            alpha_sb = singles.tile([P, N_OTILES], f32)
            alpha_dram = dramp.tile([N_OTILES, P], f32)
            colb = singles.tile([P, OUT_F], f32)
            biasb = None
            if with_bias:
                biasb = singles.tile([P, OUT_F], f32, tag="biasb")

            sign_emitted = 0
            for s in range(N_STILES):
                prologue_item = emit_quant(s)
                if s == 0:
                    emit_wload((0, 1, 2, 3))
                    prologue = []
                elif s == 1:
                    emit_wload((4, 5, 6, 7))
                prologue.append(prologue_item)
                # one |w| row-sum per s-tile, s=0..7
                if s < N_OTILES:
                    emit_wabs(s)
                if s == WPREP_S:
                    emit_wprep_head()
                if s >= WPREP_S and sign_emitted < N_OTILES:
                    for _ in range(SIGN_PER_S):
                        if sign_emitted < N_OTILES:
                            emit_sign(sign_emitted)
                            sign_emitted += 1
                    if sign_emitted == N_OTILES:
                        w_tiles.clear()
                        _wstack.close()
                if s >= LEAD:
                    emit_matmul(s - LEAD, *prologue[s - LEAD])
            for s in range(max(0, N_STILES - LEAD), N_STILES):
                emit_matmul(s, *prologue[s])

    nc.compile()
    return nc


def _get_program(with_bias: bool):
    key = bool(with_bias)
    if key not in _prog_cache:
        _prog_cache[key] = _build_program(key)
    return _prog_cache[key]


def kernel(x: np.ndarray, weight: np.ndarray, bias: np.ndarray) -> np.ndarray:
    from concourse.bass_utils import run_bass_kernel_spmd

    B, S, in_f = x.shape
    out_f = weight.shape[0]
    assert in_f == IN_F and out_f == OUT_F and B * S == N_CORES * S_SHARD

    xf = np.ascontiguousarray(
        x.astype(np.float16, copy=False).reshape(-1, IN_F)
    )
    w = np.ascontiguousarray(weight.astype(np.float16, copy=False))
    b = np.ascontiguousarray(bias.astype(np.float32, copy=False))

    with_bias = bool(np.any(b != 0.0))
    nc = _get_program(with_bias)

    in_maps = []
    for c in range(N_CORES):
        m = {
            "x_shard": xf[c * S_SHARD : (c + 1) * S_SHARD],
            "weight": w,
        }
        if with_bias:
            m["bias"] = b
        in_maps.append(m)

    res = run_bass_kernel_spmd(nc, in_maps, core_ids=list(range(N_CORES)))
    out = np.concatenate([res.results[c]["out"] for c in range(N_CORES)], axis=0)
    return out.reshape(B, S, OUT_F).astype(np.float32, copy=False)


# revision 12
# speedup vs baseline: 1.0672x; 1.0672x over previous
"""BitLinear (4-bit activation quant + ternary weight) Trainium2 kernel.

Full computation:
    xq  = round(clip(x / max_abs(x, row) * 7)) * max_abs / 7      (per-row 4-bit quant)
    wq  = sign_thresholded(w) * mean_abs(w, row)                   (ternary weight)
    out = xq @ wq.T + bias

Strategy (8 NeuronCores, data-parallel over rows of x):
  - Shard x rows 8 ways; replicate weight.
  - x and weight ship to the device as f16 (halves the HBM read traffic; the
    quant decisions from f16 inputs keep end-to-end rel err ~1e-2, inside the
    2e-2 gate). Output is exact f32 scaling of integer PE accumulations.
  - Matmul runs on exact small integers in fp8 (q in [-8,7], sign in
    {-1,0,1}) with DoubleRow perf mode. Rounding uses the +1.5*2^23
    magic-number trick == round-half-even.
  - Engine balance per 128-row s-tile (steady state): DVE absmax+scales and
    most column-scale multiplies; Pool the magic multiply-add and 1/3 of the
    column-scale multiplies; ACT both PSUM evictions (qt and out*rowscale)
    plus out-store DMA issues; PE transposes + matmuls at full clock.
"""

import os
import sys

os.environ.setdefault("MYCRO_LOCAL_CACHE", "1")

for _p in ("/opt/trn_rl_repo", "/root/.axon_site/_ro/trn_rl_repo"):
    if os.path.isdir(_p) and _p not in sys.path:
        sys.path.insert(0, _p)

import numpy as np

N_CORES = 8
S_SHARD = 4096
IN_F = 1024
OUT_F = 1024
P = 128
N_STILES = S_SHARD // P  # 32
N_KTILES = IN_F // P  # 8
N_OTILES = OUT_F // P  # 8
MM_N = 512
N_OHALF = OUT_F // MM_N  # 2
OUT_B = 4  # s-tiles per output store (2 MiB transfers)

MAGIC = 12582912.0
EPS = 1e-06

_prog_cache = {}


def _build_program(with_bias: bool):
    import concourse.bass as bass
    import concourse.mybir as mybir
    import concourse.tile as tile
    from concourse import bacc, bass_isa
    from concourse.masks import make_identity

    f32 = mybir.dt.float32
    f16 = mybir.dt.float16
    bf16 = mybir.dt.bfloat16
    f8 = mybir.dt.float8e4
    Alu = mybir.AluOpType
    Act = mybir.ActivationFunctionType

    nc = bacc.Bacc("TRN2", target_bir_lowering=False, debug=False)

    x_in = nc.dram_tensor("x_shard", [S_SHARD, IN_F], f16, kind="ExternalInput")
    w_in = nc.dram_tensor("weight", [OUT_F, IN_F], f16, kind="ExternalInput")
    if with_bias:
        b_in = nc.dram_tensor("bias", [OUT_F], f32, kind="ExternalInput")
    out_d = nc.dram_tensor("out", [S_SHARD, OUT_F], f32, kind="ExternalOutput")

    WPREP_S = int(os.environ.get("KWPREP", "8"))
    SIGN_PER_S = int(os.environ.get("KSIGNPS", "2"))
    LEAD = int(os.environ.get("KLEAD", "11"))
    XBUFS = int(os.environ.get("KXBUFS", "12"))
    OBUFS = int(os.environ.get("KOBUFS", "4"))

    with tile.TileContext(nc) as tc:
        from contextlib import ExitStack as _ES

        _wstack = _ES()
        with (
            tc.tile_pool(name="singles", bufs=1) as singles,
            tc.tile_pool(name="wtmp", bufs=2) as wtmp,
            tc.tile_pool(name="signp", bufs=2) as signp,
            tc.tile_pool(name="xp", bufs=XBUFS) as xp,
            tc.tile_pool(name="tp", bufs=3) as tp,
            tc.tile_pool(name="fevp", bufs=3) as fevp,
            tc.tile_pool(name="qtp", bufs=LEAD + 3) as qtp,
            tc.tile_pool(name="outp", bufs=OBUFS) as outp,
            tc.tile_pool(name="stats", bufs=8) as stats,
            tc.tile_pool(name="ma7p", bufs=N_STILES + 1) as ma7p,
            tc.tile_pool(name="tpsum", bufs=2, space="PSUM") as tpsum,
            tc.tile_pool(name="mpsum", bufs=2, space="PSUM") as mpsum,
            tc.tile_pool(name="dramp", bufs=1, space="DRAM") as dramp,
        ):
            identity = singles.tile([P, P], bf16)
            make_identity(nc, identity)
            identity_f = singles.tile([P, P], f32)
            make_identity(nc, identity_f)

            magneg = singles.tile([P, 1], f32)
            nc.vector.memset(magneg, -MAGIC)
            neg1 = singles.tile([P, 1], f32)
            nc.vector.memset(neg1, -1.0)

            signT8 = singles.tile([P, N_KTILES, OUT_F], f8)
            alpha_raw = singles.tile([P, N_OTILES], f32)

            wpool = _wstack.enter_context(tc.tile_pool(name="wpool", bufs=8))
            w_tiles = []
            for j in range(N_OTILES):
                w_t = wpool.tile([P, IN_F], f16, tag="w")
                w_tiles.append(w_t)

            def emit_wload(js):
                for j in js:
                    if j < N_OTILES:
                        nc.sync.dma_start(
                            out=w_tiles[j], in_=w_in[j * P : (j + 1) * P, :]
                        )

            def emit_wabs(j):
                nc.vector.tensor_reduce(
                    out=alpha_raw[:, j : j + 1],
                    in_=w_tiles[j],
                    axis=mybir.AxisListType.X,
                    op=Alu.add,
                    apply_absolute_value=True,
                )

            x_pairs = {}

            def emit_quant(s):
                if s % 2 == 0:
                    x2 = xp.tile([P, 2, IN_F], f16, tag="x")
                    if s == 0:
                        for g in range(2):
                            nc.sync.dma_start(
                                out=x2[:, g, :],
                                in_=x_in[(s + g) * P : (s + g + 1) * P, :],
                            )
                    else:
                        nc.sync.dma_start(
                            out=x2,
                            in_=x_in[s * P : (s + 2) * P, :].rearrange(
                                "(two p) f -> p two f", p=P
                            ),
                        )
                    x_pairs[s] = x2
                    x_t = x2[:, 0, :]
                else:
                    x_t = x_pairs.pop(s - 1)[:, 1, :]
                ma = stats.tile([P, 1], f32, tag="ma")
                nc.vector.tensor_reduce(
                    out=ma,
                    in_=x_t,
                    axis=mybir.AxisListType.X,
                    op=Alu.max,
                    apply_absolute_value=True,
                )
                ma7 = ma7p.tile([P, 1], f32, tag="ma7")
                nc.vector.tensor_scalar(
                    out=ma7,
                    in0=ma,
                    scalar1=float(1.0 / 7.0),
                    scalar2=float(EPS / 7.0),
                    op0=Alu.mult,
                    op1=Alu.max,
                )
                inv = stats.tile([P, 1], f32, tag="inv")
                nc.vector.reciprocal(out=inv, in_=ma7)
                # t = x*inv + MAGIC (f32; fraction now rounded half-to-even)
                # Pool while the pipeline fills (it has no eviction work
                # yet); DVE 2x-mode (~594 ns) in steady state.
                t_t = tp.tile([P, IN_F], f32, tag="t")
                b_eng = nc.gpsimd if s < LEAD else nc.vector
                b_eng.tensor_scalar(
                    out=t_t,
                    in0=x_t,
                    scalar1=inv,
                    scalar2=MAGIC,
                    op0=Alu.mult,
                    op1=Alu.add,
                )
                qt_ps = tpsum.tile([P, IN_F], f32, tag="tps")
                for k in range(N_KTILES):
                    nc.tensor.transpose(
                        qt_ps[:, k * P : (k + 1) * P],
                        t_t[:, k * P : (k + 1) * P],
                        identity_f,
                    )
                qt_sb = qtp.tile([P, N_KTILES, P], f8, tag="qt")
                nc.scalar.activation(
                    out=qt_sb.rearrange("p k c -> p (k c)"),
                    in_=qt_ps,
                    func=Act.Identity,
                    bias=magneg,
                    scale=1.0,
                )
                return ma7, qt_sb

            out_blocks = {}

            def emit_matmul(s, ma7, qt_sb):
                sb = (s // OUT_B) * OUT_B
                if s % OUT_B == 0:
                    ob = outp.tile([P, OUT_B, OUT_F], f32, tag="o")
                    out_blocks[sb] = ob
                else:
                    ob = out_blocks[sb]
                out_sb = ob[:, s % OUT_B, :]
                ps = mpsum.tile([P, OUT_F], f32, tag="mm")
                for h in range(N_OHALF):
                    for t in range(N_KTILES // 2):
                        nc.tensor.matmul(
                            ps[:, h * MM_N : (h + 1) * MM_N],
                            lhsT=qt_sb[:, 2 * t : 2 * t + 2, :],
                            rhs=signT8[
                                :, 2 * t : 2 * t + 2, h * MM_N : (h + 1) * MM_N
                            ],
                            start=(t == 0),
                            stop=(t == N_KTILES // 2 - 1),
                            perf_mode=mybir.MatmulPerfMode.DoubleRow,
                        )
                # out = (S * rowscale) * colscale: ACT evicts PSUM with the
                # per-row scale (GPSIMD can't read PSUM), then the per-column
                # multiply runs on DVE (2 of 3 tiles) or Pool (1 of 3).
                fev = fevp.tile([P, OUT_F], f32, tag="fev")
                nc.scalar.activation(
                    out=fev, in_=ps, func=Act.Identity, scale=ma7
                )
                nc.gpsimd.tensor_tensor(
                    out=out_sb, in0=fev, in1=colb, op=Alu.mult
                )
                if with_bias:
                    nc.gpsimd.tensor_tensor(
                        out=out_sb, in0=out_sb, in1=biasb, op=Alu.add
                    )
                if s % OUT_B == OUT_B - 1:
                    nc.sync.dma_start(
                        out=out_d[sb * P : (s + 1) * P, :].rearrange(
                            "(b p) f -> p b f", p=P
                        ),
                        in_=out_blocks.pop(sb),
                    )

            def emit_wprep_head():
                # global threshold = 0.05 * mean(|w|)
                g0 = stats.tile([P, 1], f32, tag="g0")
                nc.vector.tensor_reduce(
                    out=g0, in_=alpha_raw, axis=mybir.AxisListType.X, op=Alu.add
                )
                g1 = stats.tile([P, 1], f32, tag="g1")
                nc.gpsimd.partition_all_reduce(
                    out_ap=g1, in_ap=g0, channels=P, reduce_op=bass_isa.ReduceOp.add
                )
                nc.vector.tensor_scalar(
                    out=thr,
                    in0=g1,
                    scalar1=float(0.05 / (OUT_F * IN_F)),
                    scalar2=None,
                    op0=Alu.mult,
                )
                nc.vector.tensor_scalar(
                    out=nthr, in0=thr, scalar1=-1.0, scalar2=None, op0=Alu.mult
                )
                nc.vector.tensor_scalar(
                    out=alpha_sb,
                    in0=alpha_raw,
                    scalar1=float(1.0 / IN_F),
                    scalar2=None,
                    op0=Alu.mult,
                )
                # column scale alpha broadcast to all partitions via DRAM bounce
                nc.sync.dma_start(
                    out=alpha_dram.rearrange("j p -> p j"), in_=alpha_sb
                )
                alpha_flat = alpha_dram.rearrange("j p -> (j p)")
                bcast_src = bass.AP(
                    tensor=alpha_flat.tensor,
                    offset=alpha_flat.offset,
                    ap=[[0, P]] + list(alpha_flat.ap),
                )
                nc.sync.dma_start(out=colb, in_=bcast_src)
                if with_bias:
                    bias_src = bass.AP(
                        tensor=b_in.tensor
                        if hasattr(b_in, "tensor")
                        else b_in[:].tensor,
                        offset=b_in[:].offset,
                        ap=[[0, P]] + list(b_in[:].ap),
                    )
                    nc.sync.dma_start(out=biasb, in_=bias_src)

            def emit_sign(j):
                # ternary sign: sign = (w >= thr) + (w > -thr) - 1.
                # The two comparisons are cheap 4x-mode DVE tensor_scalars;
                # the add happens for free in PSUM (accumulating transposes)
                # and the -1 rides the eviction bias.
                a_cmp = wtmp.tile([P, IN_F], bf16, tag="tmp")
                nc.gpsimd.tensor_scalar(
                    out=a_cmp,
                    in0=w_tiles[j],
                    scalar1=nthr,
                    scalar2=None,
                    op0=Alu.is_gt,
                )
                b_cmp = signp.tile([P, IN_F], bf16, tag="sgn")
                nc.vector.tensor_scalar(
                    out=b_cmp,
                    in0=w_tiles[j],
                    scalar1=thr,
                    scalar2=None,
                    op0=Alu.is_ge,
                )
                ps = tpsum.tile([P, IN_F], f32, tag="tps")
                for k in range(N_KTILES):
                    nc.tensor.matmul(
                        ps[:, k * P : (k + 1) * P],
                        lhsT=a_cmp[:, k * P : (k + 1) * P],
                        rhs=identity,
                        start=True,
                        stop=False,
                    )
                    nc.tensor.matmul(
                        ps[:, k * P : (k + 1) * P],
                        lhsT=b_cmp[:, k * P : (k + 1) * P],
                        rhs=identity,
                        start=False,
                        stop=True,
                    )
                if j % 2 == 0:
                    nc.scalar.activation(
                        out=signT8[:, :, j * P : (j + 1) * P],
                        in_=ps.rearrange("p (k c) -> p k c", k=N_KTILES),
                        func=Act.Identity,
                        bias=neg1,
                    )
                else:
                    nc.vector.tensor_scalar(
                        out=signT8[:, :, j * P : (j + 1) * P],
                        in0=ps.rearrange("p (k c) -> p k c", k=N_KTILES),
                        scalar1=-1.0,
                        scalar2=None,
                        op0=Alu.add,
                    )

            thr = singles.tile([P, 1], f32)
            nthr = singles.tile([P, 1], f32)
            alpha_sb = singles.tile([P, N_OTILES], f32)
            alpha_dram = dramp.tile([N_OTILES, P], f32)
            colb = singles.tile([P, OUT_F], f32)
            biasb = None
            if with_bias:
                biasb = singles.tile([P, OUT_F], f32, tag="biasb")

            sign_emitted = 0
            for s in range(N_STILES):
                prologue_item = emit_quant(s)
                if s == 0:
                    emit_wload((0, 1, 2, 3))
                    prologue = []
                elif s == 1:
                    emit_wload((4, 5, 6, 7))
                prologue.append(prologue_item)
                # one |w| row-sum per s-tile, s=0..7
                if s < N_OTILES:
                    emit_wabs(s)
                if s == WPREP_S:
                    emit_wprep_head()
                if s >= WPREP_S and sign_emitted < N_OTILES:
                    for _ in range(SIGN_PER_S):
                        if sign_emitted < N_OTILES:
                            emit_sign(sign_emitted)
                            sign_emitted += 1
                    if sign_emitted == N_OTILES:
                        w_tiles.clear()
                        _wstack.close()
                if s >= LEAD:
                    emit_matmul(s - LEAD, *prologue[s - LEAD])
            for s in range(max(0, N_STILES - LEAD), N_STILES):
                emit_matmul(s, *prologue[s])

    nc.compile()
    return nc


def _get_program(with_bias: bool):
    key = bool(with_bias)
    if key not in _prog_cache:
        _prog_cache[key] = _build_program(key)
    return _prog_cache[key]


def kernel(x: np.ndarray, weight: np.ndarray, bias: np.ndarray) -> np.ndarray:
    from concourse.bass_utils import run_bass_kernel_spmd

    B, S, in_f = x.shape
    out_f = weight.shape[0]
    assert in_f == IN_F and out_f == OUT_F and B * S == N_CORES * S_SHARD

    xf = np.ascontiguousarray(
        x.astype(np.float16, copy=False).reshape(-1, IN_F)
    )
    w = np.ascontiguousarray(weight.astype(np.float16, copy=False))
    b = np.ascontiguousarray(bias.astype(np.float32, copy=False))

    with_bias = bool(np.any(b != 0.0))
    nc = _get_program(with_bias)

    in_maps = []
    for c in range(N_CORES):
        m = {
            "x_shard": xf[c * S_SHARD : (c + 1) * S_SHARD],
            "weight": w,
        }
        if with_bias:
            m["bias"] = b
        in_maps.append(m)

    res = run_bass_kernel_spmd(nc, in_maps, core_ids=list(range(N_CORES)))
    out = np.concatenate([res.results[c]["out"] for c in range(N_CORES)], axis=0)
    return out.reshape(B, S, OUT_F).astype(np.float32, copy=False)


# revision 14
# speedup vs baseline: 1.1734x; 1.0995x over previous
"""BitLinear (4-bit activation quant + ternary weight) Trainium2 kernel.

Full computation:
    xq  = round(clip(x / max_abs(x, row) * 7)) * max_abs / 7      (per-row 4-bit quant)
    wq  = sign_thresholded(w) * mean_abs(w, row)                   (ternary weight)
    out = xq @ wq.T + bias

Strategy (8 NeuronCores, data-parallel over rows of x):
  - Shard x rows 8 ways; replicate weight.
  - x and weight ship to the device as f16 (halves the HBM read traffic; the
    quant decisions from f16 inputs keep end-to-end rel err ~1e-2, inside the
    2e-2 gate). Output is exact f32 scaling of integer PE accumulations.
  - Matmul runs on exact small integers in fp8 (q in [-8,7], sign in
    {-1,0,1}) with DoubleRow perf mode. Rounding uses the +1.5*2^23
    magic-number trick == round-half-even.
  - Engine balance per 128-row s-tile (steady state): DVE absmax+scales and
    most column-scale multiplies; Pool the magic multiply-add and 1/3 of the
    column-scale multiplies; ACT both PSUM evictions (qt and out*rowscale)
    plus out-store DMA issues; PE transposes + matmuls at full clock.
"""

import os
import sys

os.environ.setdefault("MYCRO_LOCAL_CACHE", "1")

for _p in ("/opt/trn_rl_repo", "/root/.axon_site/_ro/trn_rl_repo"):
    if os.path.isdir(_p) and _p not in sys.path:
        sys.path.insert(0, _p)

import numpy as np

N_CORES = 8
S_SHARD = 4096
IN_F = 1024
OUT_F = 1024
P = 128
N_STILES = S_SHARD // P  # 32
N_KTILES = IN_F // P  # 8
N_OTILES = OUT_F // P  # 8
MM_N = 512
N_OHALF = OUT_F // MM_N  # 2
OUT_B = 2  # s-tiles per output store (1 MiB transfers)

MAGIC = 12582912.0
EPS = 1e-06

_prog_cache = {}


def _build_program(with_bias: bool):
    import concourse.bass as bass
    import concourse.mybir as mybir
    import concourse.tile as tile
    from concourse import bacc, bass_isa
    from concourse.masks import make_identity

    f32 = mybir.dt.float32
    f16 = mybir.dt.float16
    bf16 = mybir.dt.bfloat16
    f8 = mybir.dt.float8e4
    Alu = mybir.AluOpType
    Act = mybir.ActivationFunctionType

    nc = bacc.Bacc("TRN2", target_bir_lowering=False, debug=False)

    x_in = nc.dram_tensor("x_shard", [S_SHARD, IN_F], f16, kind="ExternalInput")
    w_in = nc.dram_tensor("weight", [OUT_F, IN_F], f16, kind="ExternalInput")
    if with_bias:
        b_in = nc.dram_tensor("bias", [OUT_F], f32, kind="ExternalInput")
    out_d = nc.dram_tensor("out", [S_SHARD, OUT_F], f32, kind="ExternalOutput")

    WPREP_S = int(os.environ.get("KWPREP", "8"))
    SIGN_PER_S = int(os.environ.get("KSIGNPS", "2"))
    LEAD = int(os.environ.get("KLEAD", "13"))
    XBUFS = int(os.environ.get("KXBUFS", "12"))
    OBUFS = int(os.environ.get("KOBUFS", "4"))

    with tile.TileContext(nc) as tc:
        from contextlib import ExitStack as _ES

        _wstack = _ES()
        with (
            tc.tile_pool(name="singles", bufs=1) as singles,
            tc.tile_pool(name="wtmp", bufs=2) as wtmp,
            tc.tile_pool(name="signp", bufs=2) as signp,
            tc.tile_pool(name="xp", bufs=XBUFS) as xp,
            tc.tile_pool(name="tp", bufs=3) as tp,
            tc.tile_pool(name="fevp", bufs=3) as fevp,
            tc.tile_pool(name="qtp", bufs=LEAD + 3) as qtp,
            tc.tile_pool(name="outp", bufs=OBUFS) as outp,
            tc.tile_pool(name="stats", bufs=8) as stats,
            tc.tile_pool(name="ma7p", bufs=N_STILES + 1) as ma7p,
            tc.tile_pool(name="tpsum", bufs=2, space="PSUM") as tpsum,
            tc.tile_pool(name="mpsum", bufs=2, space="PSUM") as mpsum,
            tc.tile_pool(name="dramp", bufs=1, space="DRAM") as dramp,
        ):
            identity = singles.tile([P, P], bf16)
            make_identity(nc, identity)
            identity_f = singles.tile([P, P], f32)
            make_identity(nc, identity_f)

            magneg = singles.tile([P, 1], f32)
            nc.vector.memset(magneg, -MAGIC)
            magpos = singles.tile([P, 1], f32)
            nc.vector.memset(magpos, MAGIC)
            neg1 = singles.tile([P, 1], f32)
            nc.vector.memset(neg1, -1.0)

            signT8 = singles.tile([P, N_KTILES, OUT_F], f8)
            alpha_raw = singles.tile([P, N_OTILES], f32)

            wpool = _wstack.enter_context(tc.tile_pool(name="wpool", bufs=8))
            w_tiles = []
            for j in range(N_OTILES):
                w_t = wpool.tile([P, IN_F], f16, tag="w")
                w_tiles.append(w_t)

            def emit_wload(js):
                for j in js:
                    if j < N_OTILES:
                        nc.sync.dma_start(
                            out=w_tiles[j], in_=w_in[j * P : (j + 1) * P, :]
                        )

            def emit_wabs(j):
                nc.vector.tensor_reduce(
                    out=alpha_raw[:, j : j + 1],
                    in_=w_tiles[j],
                    axis=mybir.AxisListType.X,
                    op=Alu.add,
                    apply_absolute_value=True,
                )

            x_pairs = {}

            def emit_quant(s):
                if s % 2 == 0:
                    x2 = xp.tile([P, 2, IN_F], f16, tag="x")
                    if s == 0:
                        for g in range(2):
                            nc.sync.dma_start(
                                out=x2[:, g, :],
                                in_=x_in[(s + g) * P : (s + g + 1) * P, :],
                            )
                    else:
                        nc.sync.dma_start(
                            out=x2,
                            in_=x_in[s * P : (s + 2) * P, :].rearrange(
                                "(two p) f -> p two f", p=P
                            ),
                        )
                    x_pairs[s] = x2
                    x_t = x2[:, 0, :]
                else:
                    x_t = x_pairs.pop(s - 1)[:, 1, :]
                ma = stats.tile([P, 1], f32, tag="ma")
                nc.vector.tensor_reduce(
                    out=ma,
                    in_=x_t,
                    axis=mybir.AxisListType.X,
                    op=Alu.max,
                    apply_absolute_value=True,
                )
                ma7 = ma7p.tile([P, 1], f32, tag="ma7")
                nc.vector.tensor_scalar(
                    out=ma7,
                    in0=ma,
                    scalar1=float(1.0 / 7.0),
                    scalar2=float(EPS / 7.0),
                    op0=Alu.mult,
                    op1=Alu.max,
                )
                inv = stats.tile([P, 1], f32, tag="inv")
                nc.vector.reciprocal(out=inv, in_=ma7)
                # t = x*inv + MAGIC (f32; fraction now rounded half-to-even)
                # Pool while the pipeline fills (it has no eviction work
                # yet); DVE 2x-mode (~594 ns) in steady state.
                t_t = tp.tile([P, IN_F], f32, tag="t")
                if s < LEAD or s % 2 == 0:
                    nc.gpsimd.tensor_scalar(
                        out=t_t,
                        in0=x_t,
                        scalar1=inv,
                        scalar2=MAGIC,
                        op0=Alu.mult,
                        op1=Alu.add,
                    )
                else:
                    nc.scalar.activation(
                        out=t_t,
                        in_=x_t,
                        func=Act.Identity,
                        bias=magpos,
                        scale=inv,
                    )
                qt_ps = tpsum.tile([P, IN_F], f32, tag="tps")
                for k in range(N_KTILES):
                    nc.tensor.transpose(
                        qt_ps[:, k * P : (k + 1) * P],
                        t_t[:, k * P : (k + 1) * P],
                        identity_f,
                    )
                qt_sb = qtp.tile([P, N_KTILES, P], f8, tag="qt")
                nc.scalar.activation(
                    out=qt_sb.rearrange("p k c -> p (k c)"),
                    in_=qt_ps,
                    func=Act.Identity,
                    bias=magneg,
                    scale=1.0,
                )
                return ma7, qt_sb

            out_blocks = {}

            def emit_matmul(s, ma7, qt_sb):
                sb = (s // OUT_B) * OUT_B
                if s % OUT_B == 0:
                    ob = outp.tile([P, OUT_B, OUT_F], f32, tag="o")
                    out_blocks[sb] = ob
                else:
                    ob = out_blocks[sb]
                out_sb = ob[:, s % OUT_B, :]
                ps = mpsum.tile([P, OUT_F], f32, tag="mm")
                for h in range(N_OHALF):
                    for t in range(N_KTILES // 2):
                        nc.tensor.matmul(
                            ps[:, h * MM_N : (h + 1) * MM_N],
                            lhsT=qt_sb[:, 2 * t : 2 * t + 2, :],
                            rhs=signT8[
                                :, 2 * t : 2 * t + 2, h * MM_N : (h + 1) * MM_N
                            ],
                            start=(t == 0),
                            stop=(t == N_KTILES // 2 - 1),
                            perf_mode=mybir.MatmulPerfMode.DoubleRow,
                        )
                # out = (S * rowscale) * colscale: ACT evicts PSUM with the
                # per-row scale (GPSIMD can't read PSUM), then the per-column
                # multiply runs on DVE (2 of 3 tiles) or Pool (1 of 3).
                fev = fevp.tile([P, OUT_F], f32, tag="fev")
                if s % 4 == 1:
                    nc.vector.tensor_scalar(
                        out=fev, in0=ps, scalar1=ma7, scalar2=None, op0=Alu.mult
                    )
                else:
                    nc.scalar.activation(
                        out=fev, in_=ps, func=Act.Identity, scale=ma7
                    )
                f2_eng = nc.vector if s % 2 == 1 else nc.gpsimd
                f2_eng.tensor_tensor(
                    out=out_sb, in0=fev, in1=colb, op=Alu.mult
                )
                if with_bias:
                    nc.gpsimd.tensor_tensor(
                        out=out_sb, in0=out_sb, in1=biasb, op=Alu.add
                    )
                if s % OUT_B == OUT_B - 1:
                    nc.sync.dma_start(
                        out=out_d[sb * P : (s + 1) * P, :].rearrange(
                            "(b p) f -> p b f", p=P
                        ),
                        in_=out_blocks.pop(sb),
                    )

            def emit_wprep_head():
                # global threshold = 0.05 * mean(|w|)
                g0 = stats.tile([P, 1], f32, tag="g0")
                nc.vector.tensor_reduce(
                    out=g0, in_=alpha_raw, axis=mybir.AxisListType.X, op=Alu.add
                )
                g1 = stats.tile([P, 1], f32, tag="g1")
                nc.gpsimd.partition_all_reduce(
                    out_ap=g1, in_ap=g0, channels=P, reduce_op=bass_isa.ReduceOp.add
                )
                nc.vector.tensor_scalar(
                    out=thr,
                    in0=g1,
                    scalar1=float(0.05 / (OUT_F * IN_F)),
                    scalar2=None,
                    op0=Alu.mult,
                )
                nc.vector.tensor_scalar(
                    out=nthr, in0=thr, scalar1=-1.0, scalar2=None, op0=Alu.mult
                )
                nc.vector.tensor_scalar(
                    out=alpha_sb,
                    in0=alpha_raw,
                    scalar1=float(1.0 / IN_F),
                    scalar2=None,
                    op0=Alu.mult,
                )
                # column scale alpha broadcast to all partitions via DRAM bounce
                nc.sync.dma_start(
                    out=alpha_dram.rearrange("j p -> p j"), in_=alpha_sb
                )
                alpha_flat = alpha_dram.rearrange("j p -> (j p)")
                bcast_src = bass.AP(
                    tensor=alpha_flat.tensor,
                    offset=alpha_flat.offset,
                    ap=[[0, P]] + list(alpha_flat.ap),
                )
                nc.sync.dma_start(out=colb, in_=bcast_src)
                if with_bias:
                    bias_src = bass.AP(
                        tensor=b_in.tensor
                        if hasattr(b_in, "tensor")
                        else b_in[:].tensor,
                        offset=b_in[:].offset,
                        ap=[[0, P]] + list(b_in[:].ap),
                    )
                    nc.sync.dma_start(out=biasb, in_=bias_src)

            def emit_sign(j):
                # ternary sign: sign = (w >= thr) + (w > -thr) - 1.
                # The two comparisons are cheap 4x-mode DVE tensor_scalars;
                # the add happens for free in PSUM (accumulating transposes)
                # and the -1 rides the eviction bias.
                a_cmp = wtmp.tile([P, IN_F], bf16, tag="tmp")
                nc.vector.tensor_scalar(
                    out=a_cmp,
                    in0=w_tiles[j],
                    scalar1=nthr,
                    scalar2=None,
                    op0=Alu.is_gt,
                )
                b_cmp = signp.tile([P, IN_F], bf16, tag="sgn")
                nc.vector.tensor_scalar(
                    out=b_cmp,
                    in0=w_tiles[j],
                    scalar1=thr,
                    scalar2=None,
                    op0=Alu.is_ge,
                )
                ps = mpsum.tile([P, IN_F], f32, tag="mm")
                for k in range(N_KTILES):
                    nc.tensor.matmul(
                        ps[:, k * P : (k + 1) * P],
                        lhsT=a_cmp[:, k * P : (k + 1) * P],
                        rhs=identity,
                        start=True,
                        stop=False,
                    )
                    nc.tensor.matmul(
                        ps[:, k * P : (k + 1) * P],
                        lhsT=b_cmp[:, k * P : (k + 1) * P],
                        rhs=identity,
                        start=False,
                        stop=True,
                    )
                if j % 2 == 0:
                    nc.scalar.activation(
                        out=signT8[:, :, j * P : (j + 1) * P],
                        in_=ps.rearrange("p (k c) -> p k c", k=N_KTILES),
                        func=Act.Identity,
                        bias=neg1,
                    )
                else:
                    nc.vector.tensor_scalar(
                        out=signT8[:, :, j * P : (j + 1) * P],
                        in0=ps.rearrange("p (k c) -> p k c", k=N_KTILES),
                        scalar1=-1.0,
                        scalar2=None,
                        op0=Alu.add,
                    )

            thr = singles.tile([P, 1], f32)
            nthr = singles.tile([P, 1], f32)
            alpha_sb = singles.tile([P, N_OTILES], f32)
            alpha_dram = dramp.tile([N_OTILES, P], f32)
            colb = singles.tile([P, OUT_F], f32)
            biasb = None
            if with_bias:
                biasb = singles.tile([P, OUT_F], f32, tag="biasb")

            sign_emitted = 0
            for s in range(N_STILES):
                prologue_item = emit_quant(s)
                if s == 0:
                    emit_wload((0, 1, 2, 3))
                    prologue = []
                elif s == 1:
                    emit_wload((4, 5, 6, 7))
                prologue.append(prologue_item)
                # one |w| row-sum per s-tile, s=0..7
                if s < N_OTILES:
                    emit_wabs(s)
                if s == WPREP_S:
                    emit_wprep_head()
                if s >= WPREP_S and sign_emitted < N_OTILES:
                    for _ in range(SIGN_PER_S):
                        if sign_emitted < N_OTILES:
                            emit_sign(sign_emitted)
                            sign_emitted += 1
                    if sign_emitted == N_OTILES:
                        w_tiles.clear()
                        _wstack.close()
                if s >= LEAD:
                    emit_matmul(s - LEAD, *prologue[s - LEAD])
            for s in range(max(0, N_STILES - LEAD), N_STILES):
                emit_matmul(s, *prologue[s])

    nc.compile()
    return nc


def _get_program(with_bias: bool):
    key = bool(with_bias)
    if key not in _prog_cache:
        _prog_cache[key] = _build_program(key)
    return _prog_cache[key]


def kernel(x: np.ndarray, weight: np.ndarray, bias: np.ndarray) -> np.ndarray:
    from concourse.bass_utils import run_bass_kernel_spmd

    B, S, in_f = x.shape
    out_f = weight.shape[0]
    assert in_f == IN_F and out_f == OUT_F and B * S == N_CORES * S_SHARD

    xf = np.ascontiguousarray(
        x.astype(np.float16, copy=False).reshape(-1, IN_F)
    )
    w = np.ascontiguousarray(weight.astype(np.float16, copy=False))
    b = np.ascontiguousarray(bias.astype(np.float32, copy=False))

    with_bias = bool(np.any(b != 0.0))
    nc = _get_program(with_bias)

    in_maps = []
    for c in range(N_CORES):
        m = {
            "x_shard": xf[c * S_SHARD : (c + 1) * S_SHARD],
            "weight": w,
        }
        if with_bias:
            m["bias"] = b
        in_maps.append(m)

    res = run_bass_kernel_spmd(nc, in_maps, core_ids=list(range(N_CORES)))
    out = np.concatenate([res.results[c]["out"] for c in range(N_CORES)], axis=0)
    return out.reshape(B, S, OUT_F).astype(np.float32, copy=False)
